# revision 26
# baseline (speedup 1.0000x reference)
"""Trainium2 kernel for MinibatchDiscrimination.

reference:
    M = einsum('ni,ibk->nbk', x, T)            # (256, 256, 16)
    l1[n,m,b] = sum_k |M[n,b,k] - M[m,b,k]|
    out[m,b]  = sum_n exp(-l1[n,m,b]) - 1      # (256, 256)
    return concat([x, out], axis=1)            # (256, 1280)

Sharding: tensor-parallel over the B_extra=256 feature dim -> 32 features
per core, no collectives. Each core computes out[:, shard] as [32, 256]
(batch on partitions), host transposes and concatenates with x.

Per-core dataflow:
  MT[(k,b), m] = M[m, b, k] via PE (bf16, k-major so every 128-partition
  chunk maps partition p -> b = p%32 with the same selector stationary).
  A PE warm-up spin runs during the input DMAs to flip HAM to 2.4 GHz.

  Row part (the G4 widest row-groups, pairs of 4 n-rows): ACT computes
  |0.5 d| via Abs with per-partition bias; selector matmuls (weight 2)
  contract each chunk into psl1[(j,b), m]; an L-poison matmul zeroes the
  diagonal corner; one batched exp per group with accum_out giving the
  transposed-half sums; one stacked-identity matmul row-accumulates.

  Diagonal part (pairs with min(n,m) >= 4*G4): uses
  sum_k |d_k| = 2*sum_k max(M[n,k], M[m,k]) - SS[n] - SS[m].
  One DVE tensor_tensor(max) per (8-diagonal block, chunk) computes
  r[p, d, i] = max(M[p, i+dt+d], M[p, i]) via an overlapping-window AP
  and a stride-0 broadcast AP (2x mode, no per-partition scalar).
  psl1[(dj,b), i] accumulates -SS[i] (one i4h matmul), -SS[i+delta+j]
  (ONE identity matmul over the row-pre-shifted sthi4 tile), and the
  selector contractions; mt_b/sthi pad columns poison out-of-range reads
  so their exp terms vanish. exp has no bias and no accum_out; one
  row-acc matmul adds the i-side, and the (i+delta)-side goes into a
  skewed accumulator acc_sk[(j,b), i+4u] with ONE identity matmul per
  group (the skew offset 4u is row-uniform), unscrambled once at the
  end by four block matmuls (m = t + 1 + j).

  Epilogue: accn4 (row-group accum_outs) is unscrambled into acc via
  four block matmuls; acc is copied out and DMA'd as [32, 256].

  A post-pass dedupes consecutive identical LDWEIGHTS (walrus ldw-opt is
  broken here); matmuls are issued stationary-major across group pairs
  so the dedupe collapses most weight loads. Units are software-pipelined
  depth-3 so DVE/ACT run ahead of PE and PE stays warm.
"""

import sys

sys.path.insert(0, "/opt/trn_rl_repo")

import os
import numpy as np
import ml_dtypes

G4 = int(os.environ.get("MBD_G4", "6"))        # row groups < G4: ACT abs path
M_LO = 4 * G4                                   # diag part covers min(n,m) >= M_LO
S = 256 - M_LO                                  # diag triangle side
PADW = 12                                       # mt_b pad columns (poison 64.0)
USE_MAX = int(os.environ.get("MBD_MAX", "1"))  # single-op max trick vs dual-op relu
R_BUFS = int(os.environ.get("MBD_R_BUFS", "14"))
E_BUFS = int(os.environ.get("MBD_E_BUFS", "10"))
PSL1_BUFS = int(os.environ.get("MBD_PSL1_BUFS", "6"))
PSMT_BUFS = int(os.environ.get("MBD_PSMT_BUFS", "2"))
POISON = 100.0

N = 256
IN_FEATURES = 1024
B_EXTRA = 256
K = 16
N_CORES = 8
B_LOCAL = B_EXTRA // N_CORES          # 32 features per core
BK = B_LOCAL * K                      # 512 = (k, b) flattened, k-major
N_CHUNKS = BK // 128                  # 4 partition chunks of (k, b)
I_CHUNKS = IN_FEATURES // 128         # 8 contraction chunks
NG = N // 4                           # 64 groups of 4 rows

_COMPILED = None


def _apply_tile_drain_patch():
    """walrus in this container caps Drain (CTRL) instructions at one sem
    wait; Tile's end-of-kernel drain carries one wait per outstanding proc.
    Split the waits across a chain of drains."""
    from concourse import mybir, tile
    from concourse.vector_clock import ScopedClock

    def _drain_and_barrier(self, tick_clock, wait_clock):
        drain_inst = self.nc.sync.drain()
        wait_clock.add_sem_waits(
            drain_inst.ins, ScopedClock({None: tick_clock.global_clock})
        )
        si = drain_inst.ins.sync_info
        if si is not None and si.on_wait and len(si.on_wait) > 1:
            waits = list(si.on_wait)
            drain_inst.ins.sync_info = mybir.SyncInfo(
                on_wait=[waits[0]], on_update=list(si.on_update or [])
            )
            for w in waits[1:]:
                d = self.nc.sync.drain()
                d.ins.sync_info = mybir.SyncInfo(on_wait=[w], on_update=[])

        self.nc.all_engine_barrier()
        assert self.sems is not None
        popped = self.nc._tile_sem_poison_stack.pop()
        assert popped is self._sem_poison
        self.nc.clear_and_free_semaphores(list(self.sems.allocated().values()))
        self.nc.all_engine_barrier()

    tile.TileContext._drain_and_barrier = _drain_and_barrier


def _split_multi_waits(nc, max_waits=1):
    """This walrus build accepts at most one sync wait per instruction.
    Hoist extra waits onto NoOp instructions inserted just before the
    offending instruction in the same engine's stream."""
    from concourse import mybir

    cnt = 0
    for blk in nc.main_func.blocks:
        insts = blk.instructions
        if not any(
            inst.sync_info is not None
            and inst.sync_info.on_wait
            and len(inst.sync_info.on_wait) > max_waits
            for inst in insts
        ):
            continue
        new_list = []
        for inst in insts:
            si = inst.sync_info
            if si is not None and si.on_wait and len(si.on_wait) > max_waits:
                waits = list(si.on_wait)
                for w in waits[:-max_waits]:
                    nop = mybir.InstNoOp(name=f"wsplit-{cnt}", ins=[], outs=[])
                    cnt += 1
                    nop.engine = inst.engine
                    nop.sync_info = mybir.SyncInfo(on_wait=[w], on_update=[])
                    new_list.append(nop)
                inst.sync_info = mybir.SyncInfo(
                    on_wait=waits[-max_waits:],
                    on_update=list(si.on_update or []),
                )
            new_list.append(inst)
        insts[:] = new_list
    return cnt


def _dedupe_ldweights(nc):
    """Drop InstLdweights identical to the previous one in the same block
    (PE weights persist across matmuls). Only drops instructions with no
    sync waits/updates, so the semaphore plan is unchanged."""
    dropped = 0
    for blk in nc.main_func.blocks:
        insts = blk.instructions
        last = None
        new_list = []
        for inst in insts:
            tn = type(inst).__name__
            if tn == "InstLdweights":
                key = (str(inst.ins[0]), str(inst.perf_mode),
                       str(inst.is_transpose), str(inst.tile_position))
                si = inst.sync_info
                clean = si is None or (not si.on_wait and not si.on_update)
                if key == last and clean:
                    dropped += 1
                    continue
                last = key
            elif tn == "InstMatmult":
                if getattr(inst, "is_transpose", None):
                    last = None
            new_list.append(inst)
        insts[:] = new_list
    return dropped


def _apply_ldw_opt_patch():
    """Let walrus dedupe back-to-back identical LDWEIGHTS (the compile
    path hardcodes --enable-ldw-opt=false; consecutive matmuls often
    reuse the same stationary here)."""
    from concourse import bass_utils

    if getattr(bass_utils, "_mbd_ldw_patched", False):
        return
    orig = bass_utils.run_command

    def patched(argv, **kw):
        argv = ["--enable-ldw-opt=true" if a == "--enable-ldw-opt=false"
                else a for a in argv]
        return orig(argv, **kw)

    bass_utils.run_command = patched
    bass_utils._mbd_ldw_patched = True


def _build():
    from concourse import bass, mybir, tile

    _apply_tile_drain_patch()
    A = mybir.AluOpType
    F32 = mybir.dt.float32
    BF16 = mybir.dt.bfloat16
    Exp = mybir.ActivationFunctionType.Exp
    Abs = mybir.ActivationFunctionType.Abs

    nc = bass.Bass()
    xt_d = nc.declare_dram_parameter("xT", [IN_FEATURES, N], BF16, isOutput=False)
    t_d = nc.declare_dram_parameter("Tsh", [IN_FEATURES, BK], BF16, isOutput=False)
    selr_d = nc.declare_dram_parameter("SELR", [128, 4 * 128], BF16, isOutput=False)
    i4h_d = nc.declare_dram_parameter("I4H", [B_LOCAL, 128], BF16, isOutput=False)
    i4v_d = nc.declare_dram_parameter("I4V", [128, B_LOCAL], BF16, isOutput=False)
    st4_d = nc.declare_dram_parameter("ST4", [4, 128], BF16, isOutput=False)
    lmov_d = nc.declare_dram_parameter("LMOV", [4, 4], BF16, isOutput=False)
    wf_d = nc.declare_dram_parameter("WF", [128, B_LOCAL], F32, isOutput=False)
    blk4b_d = nc.declare_dram_parameter("BLK4B", [B_LOCAL, 4 * 128], BF16, isOutput=False)
    blk4t_d = nc.declare_dram_parameter("BLK4T", [128, 4 * B_LOCAL], F32, isOutput=False)
    blk4tb_d = nc.declare_dram_parameter("BLK4TB", [128, 4 * B_LOCAL], BF16, isOutput=False)
    i128_d = nc.declare_dram_parameter("I128", [128, 128], BF16, isOutput=False)
    out_d = nc.declare_dram_parameter("out", [B_LOCAL, N], F32, isOutput=True)

    with tile.TileContext(nc) as tc:
        with (
            tc.tile_pool(name="const", bufs=1) as const_pool,
            tc.tile_pool(name="r", bufs=R_BUFS) as r_pool,
            tc.tile_pool(name="ra", bufs=10) as ra_pool,
            tc.tile_pool(name="e", bufs=E_BUFS) as e_pool,
            tc.tile_pool(name="psacc", bufs=1, space="PSUM") as psacc_pool,
        ):
            psmt_ctx = tc.tile_pool(name="psmt", bufs=PSMT_BUFS, space="PSUM")
            psmt_pool = psmt_ctx.__enter__()
            # ---- load inputs & stationaries ----
            xt = const_pool.tile([128, I_CHUNKS, N], BF16, tag="xt")
            nc.sync.dma_start(xt[:], xt_d.rearrange("(c p) m -> p c m", p=128))
            tsh = const_pool.tile([128, I_CHUNKS, BK], BF16, tag="tsh")
            tshr = t_d.rearrange("(c p) m -> p c m", p=128)
            for bc in range(N_CHUNKS):
                nc.sync.dma_start(tsh[:, :, 128 * bc:128 * (bc + 1)],
                                  tshr[:, :, 128 * bc:128 * (bc + 1)])
            selr = const_pool.tile([128, 4, 128], BF16, tag="selr")
            nc.sync.dma_start(selr[:], selr_d.rearrange("p (j q) -> p j q", j=4))
            i4h = const_pool.tile([B_LOCAL, 128], BF16, tag="i4h")
            nc.sync.dma_start(i4h[:], i4h_d[:])
            i4v = const_pool.tile([128, B_LOCAL], BF16, tag="i4v")
            nc.sync.dma_start(i4v[:], i4v_d[:])
            st4 = const_pool.tile([4, 128], BF16, tag="st4")
            nc.sync.dma_start(st4[:], st4_d[:])
            lmov = const_pool.tile([4, 4], BF16, tag="lmov")
            nc.sync.dma_start(lmov[:], lmov_d[:])
            wf = const_pool.tile([128, B_LOCAL], F32, tag="wf")
            nc.sync.dma_start(wf[:], wf_d[:])
            blk4b = const_pool.tile([B_LOCAL, 4, 128], BF16, tag="blk4b")
            nc.sync.dma_start(blk4b[:], blk4b_d.rearrange("p (j q) -> p j q", j=4))
            blk4t = const_pool.tile([128, 4, B_LOCAL], F32, tag="blk4t")
            nc.sync.dma_start(blk4t[:], blk4t_d.rearrange("p (j q) -> p j q", j=4))
            blk4tb = const_pool.tile([128, 4, B_LOCAL], BF16, tag="blk4tb")
            nc.sync.dma_start(blk4tb[:], blk4tb_d.rearrange("p (j q) -> p j q", j=4))
            i128 = const_pool.tile([128, 128], BF16, tag="i128")
            nc.sync.dma_start(i128[:], i128_d[:])

            # ---- PE warm-up: ~20 back-to-back matmuls on scratch while
            # the input DMAs land; flips HAM to 2.4 GHz before MT ----
            wsc = const_pool.tile([128, 512], BF16, tag="wsc")
            nc.vector.memset(wsc[:], 0.5)
            wps = psmt_pool.tile([128, 512], F32, tag="wps", bufs=1)
            for i in range(13):
                nc.tensor.matmul(wps[:], wsc[:, 0:128], wsc[:],
                                 start=(i == 0), stop=(i == 12),
                                 skip_group_check=True)

            # ---- MT per chunk: MT[(k,b), m], plus f32/bf16/neg copies ----
            mt_f = const_pool.tile([128, N_CHUNKS, N], F32, tag="mtf")
            mt_b = const_pool.tile([128, N_CHUNKS, N + PADW], BF16, tag="mtb")
            mtn_f = const_pool.tile([128, 4, N], F32, tag="mtnf")
            for c in range(N_CHUNKS):
                ps = psmt_pool.tile([128, N], F32)
                for ic in range(I_CHUNKS):
                    nc.tensor.matmul(
                        ps[:],
                        tsh[:, ic, 128 * c:128 * (c + 1)],
                        xt[:, ic, :],
                        start=(ic == 0),
                        stop=(ic == I_CHUNKS - 1),
                    )
                nc.vector.tensor_copy(mt_f[:, c, :], ps[:])
                nc.scalar.activation(
                    mt_b[:, c, 0:N], ps[:],
                    mybir.ActivationFunctionType.Copy, bias=0.0, scale=1.0)
                nc.scalar.activation(
                    mtn_f[:, c, :], ps[:],
                    mybir.ActivationFunctionType.Copy, bias=0.0, scale=-0.5)

            # pad columns: mt_b poison 64.0 so out-of-range diag reads give e=0
            nc.vector.memset(mt_b[:, :, N:], 64.0)

            # ---- SS = sum_k M over all 16 k; sthi = -SS (bf16, padded) ----
            ss_a = psmt_pool.tile([B_LOCAL, N], F32, tag="ssa", bufs=1)
            for c in range(4):
                nc.tensor.matmul(ss_a[:], wf[:], mt_f[:, c, :],
                                 start=(c == 0), stop=(c == 3))
            sthi_a = const_pool.tile([B_LOCAL, N + PADW], BF16, tag="sthia")
            nc.vector.tensor_scalar(sthi_a[:, 0:N], ss_a[:], -1.0, None, A.mult)
            nc.vector.memset(sthi_a[:, N:], -1024.0)
            # sthi4[(j,b), i] = -SS[b, i+j]: one-matmul corr-delta per group
            ps4s = psmt_pool.tile([128, N], F32, tag="ps4s", bufs=1)
            for j in range(4):
                nc.tensor.matmul(ps4s[:], blk4b[:, j, :], sthi_a[:, j:j + N],
                                 start=(j == 0), stop=(j == 3))
            sthi4 = const_pool.tile([128, N], BF16, tag="sthi4")
            nc.vector.tensor_copy(sthi4[:], ps4s[:])

            accn4 = const_pool.tile([128, NG], F32, tag="accn4")
            nc.vector.memset(accn4[:], 0.0)
            acc_ps = psacc_pool.tile([B_LOCAL, N], F32)
            nc.vector.memset(acc_ps[:], 0.0)
            acc_sk = psacc_pool.tile([128, N], F32, tag="accsk", bufs=1)
            nc.vector.memset(acc_sk[:], 0.0)

            psmt_ctx.__exit__(None, None, None)
            psl1_ctx = tc.tile_pool(name="psl1", bufs=PSL1_BUFS, space="PSUM")
            psl1_pool = psl1_ctx.__enter__()

            # ---- main loop: pairs of groups, stationary-major so the
            # ldweights dedupe pass collapses repeated stationaries ----
            from concourse.ap import AP as _AP

            def make_r_row(g):
                n0, w = 4 * g, N - 4 * g
                rt = {}
                for c in range(N_CHUNKS):
                    r = ra_pool.tile([128, 4, w], BF16, tag="ra",
                                     name=f"ra_{g}_{c}")
                    for j in range(4):
                        nc.scalar.activation(
                            r[:, j, :], mt_b[:, c, n0:N], Abs,
                            bias=mtn_f[:, c, n0 + j:n0 + j + 1], scale=0.5)
                    rt[c] = r
                return rt

            def front_row(pair, rts):
                psl1s = {}
                for g in pair:
                    psl1s[g] = psl1_pool.tile([128, N - 4 * g], F32,
                                              tag="psl1", name=f"psl1_{g}")
                for g in pair:
                    nc.tensor.matmul(psl1s[g][:, 0:4], st4[:], lmov[:],
                                     start=True, stop=False,
                                     skip_group_check=True)
                for j in range(4):
                    for g in pair:
                        for c in range(N_CHUNKS):
                            lastmm = (j == 3 and g == pair[-1] and c == 3)
                            nc.tensor.matmul(psl1s[g][:], selr[:, j, :],
                                             rts[g][c][:, j, :],
                                             start=False, stop=lastmm,
                                             skip_group_check=True)
                return psl1s

            def back_row(pair, psl1s):
                es = {}
                for g in pair:
                    e = e_pool.tile([128, N - 4 * g], BF16, tag="e",
                                    name=f"e_{g}")
                    nc.scalar.activation(e[:], psl1s[g][:], Exp,
                                         bias=0.0, scale=-1.0,
                                         accum_out=accn4[:, g:g + 1])
                    es[g] = e
                for g in pair:
                    nc.tensor.matmul(acc_ps[:, 4 * g:N], i4v[:], es[g][:],
                                     start=False, stop=False,
                                     skip_group_check=True)

            def tt_block(dt_):
                """r[c][p, d, i] = max(M[p, mlo+i+dt_+d], M[p, mlo+i])"""
                wtt = S - dt_
                rt = {}
                for c in range(N_CHUNKS):
                    r = r_pool.tile([128, 8, wtt], BF16, tag="r",
                                    name=f"r_{dt_}_{c}")
                    row = mt_b[:, c, :]
                    base = mt_b[:, c, M_LO + dt_:M_LO + dt_ + 1]
                    in0 = _AP(tensor=base.tensor, offset=base.offset,
                              ap=[list(row.ap[0]), [1, 8], [1, wtt]])
                    in1 = mt_b[:, c, M_LO:M_LO + wtt].unsqueeze(1) \
                        .broadcast_to((128, 8, wtt))
                    nc.vector.tensor_tensor(r[:], in0, in1, A.max)
                    rt[c] = r
                return rt

            def front_diag(d0s, rts, dt_):
                wtt = S - dt_
                psl1s = {}
                for d0 in d0s:
                    psl1s[d0] = psl1_pool.tile([128, S - d0], F32,
                                               tag="psl1", name=f"psd_{d0}")
                # -SS[i] for all 4 rows (start=True, covers 128 partitions)
                for d0 in d0s:
                    nc.tensor.matmul(psl1s[d0][:], i4h[:],
                                     sthi_a[:, M_LO:M_LO + S - d0],
                                     start=True, stop=False,
                                     skip_group_check=True)
                # -SS[i + delta + j] for all rows via pre-shifted sthi4
                for d0 in d0s:
                    w = S - d0
                    nc.tensor.matmul(psl1s[d0][:], i128[:],
                                     sthi4[:, M_LO + d0:M_LO + d0 + w],
                                     start=False, stop=False,
                                     skip_group_check=True)
                for j in range(4):
                    for d0 in d0s:
                        w = S - d0
                        ro = (d0 - dt_) + j
                        for c in range(N_CHUNKS):
                            lastmm = (j == 3 and d0 == d0s[-1] and c == 3)
                            mv = rts[c][:, ro, 0:w]
                            nc.tensor.matmul(psl1s[d0][:], selr[:, j, :], mv,
                                             start=False, stop=lastmm,
                                             skip_group_check=True)
                return psl1s

            def back_diag(d0s, psl1s):
                es = {}
                for d0 in d0s:
                    e = e_pool.tile([128, S - d0], BF16, tag="e",
                                    name=f"ed_{d0}")
                    nc.scalar.activation(e[:], psl1s[d0][:], Exp,
                                         bias=0.0, scale=-1.0)
                    es[d0] = e
                for d0 in d0s:
                    nc.tensor.matmul(acc_ps[:, M_LO:M_LO + S - d0], i4v[:],
                                     es[d0][:], start=False, stop=False,
                                     skip_group_check=True)
                for d0 in d0s:
                    w = S - d0
                    off = M_LO + d0 - 1
                    nc.tensor.matmul(acc_sk[:, off:off + w], i128[:],
                                     es[d0][:], start=False, stop=False,
                                     skip_group_check=True)

            # schedule: 29ish diag blocks + row pairs woven in
            row_pairs = []
            rg = list(range(G4))
            while rg:
                a = rg.pop(0)
                b = rg.pop(-1) if rg else None
                row_pairs.append([a] if b is None else [a, b])
            n_tt = (S - 2 + 8 - 1) // 8 if S > 1 else 0
            units = []
            ri = 0
            stride = max(1, n_tt // max(1, len(row_pairs)))
            for t in range(n_tt):
                units.append(("tt", t))
                if ri < len(row_pairs) and t % stride == stride // 2:
                    units.append(("row", row_pairs[ri])); ri += 1
            while ri < len(row_pairs):
                units.append(("row", row_pairs[ri])); ri += 1

            def run_back(p):
                if p[0] == "row":
                    back_row(p[1], p[2])
                else:
                    back_diag(p[1], p[2])

            pending = []
            for kind, u in units:
                if kind == "row":
                    rts = {g: make_r_row(g) for g in u}
                    psl1s = front_row(u, rts)
                    nxt = ("row", u, psl1s)
                else:
                    dt_ = 1 + 8 * u
                    d0s = [d0 for d0 in (dt_, dt_ + 4) if d0 < S]
                    rts = tt_block(dt_)
                    psl1s = front_diag(d0s, rts, dt_)
                    nxt = ("diag", d0s, psl1s)
                pending.append(nxt)
                if len(pending) > 2:
                    run_back(pending.pop(0))
            for p in pending:
                run_back(p)

            # ---- epilogue: fold skewed diag accumulator into acc ----
            acc_skb = e_pool.tile([128, N], F32, tag="accskb", bufs=1)
            nc.vector.tensor_copy(acc_skb[:], acc_sk[:])
            for j in range(4):
                wm = N - 1 - j - M_LO
                nc.tensor.matmul(acc_ps[:, M_LO + 1 + j:N], blk4t[:, j, :],
                                 acc_skb[:, M_LO:M_LO + wm],
                                 start=False, stop=False,
                                 skip_group_check=True)

            # ---- add accn4 into acc via block matmuls, store ----
            accps4 = acc_ps.rearrange("p (g r) -> p g r", r=4)
            for j in range(4):
                nc.tensor.matmul(accps4[:, :, j], blk4t[:, j, :], accn4[:],
                                 start=False, stop=(j == 3),
                                 skip_group_check=True)
            accf = e_pool.tile([B_LOCAL, N], F32, tag="accf")
            nc.vector.tensor_copy(accf[:], acc_ps[:])
            nc.sync.dma_start(out_d[:], accf[:])
            psl1_ctx.__exit__(None, None, None)

    _dedupe_ldweights(nc)
    _split_multi_waits(nc)
    return nc


def _stationaries():
    p = np.arange(128)
    selr = np.zeros((128, 4, 128), dtype=np.float32)
    for j in range(4):
        selr[p, j, 32 * j + p % 32] = 2.0
    i4h = np.zeros((B_LOCAL, 128), dtype=np.float32)
    for q in range(128):
        i4h[q % 32, q] = 1.0
    i4v = np.zeros((128, B_LOCAL), dtype=np.float32)
    i4v[p, p % 32] = 1.0
    st4 = np.zeros((4, 128), dtype=np.float32)
    st4[p // 32, p] = 1.0
    lmov = np.zeros((4, 4), dtype=np.float32)
    for r in range(4):
        lmov[r, :r + 1] = POISON
    wf = np.zeros((128, B_LOCAL), dtype=np.float32)
    wf[p, p % 32] = 1.0
    blk4 = np.zeros((B_LOCAL, 4, 128), dtype=np.float32)
    for j in range(4):
        blk4[np.arange(32), j, 32 * j + np.arange(32)] = 1.0
    blk4t = np.zeros((128, 4, B_LOCAL), dtype=np.float32)
    for j in range(4):
        blk4t[32 * j + np.arange(32), j, np.arange(32)] = 1.0
    return selr, i4h, i4v, st4, lmov, wf, blk4, blk4t


def kernel(x: np.ndarray, T: np.ndarray) -> np.ndarray:
    global _COMPILED
    from concourse.bass_utils import run_bass_kernel_spmd

    x = np.ascontiguousarray(x, dtype=np.float32)
    T = np.ascontiguousarray(T, dtype=np.float32)

    if _COMPILED is None:
        _COMPILED = _build()
    nc = _COMPILED

    in_maps = _in_maps(x, T)
    res = run_bass_kernel_spmd(nc, in_maps, core_ids=list(range(N_CORES)))

    out = np.empty((N, IN_FEATURES + B_EXTRA), dtype=np.float32)
    out[:, :IN_FEATURES] = x
    for c in range(N_CORES):
        blk = res.results[c]["out"]                      # (32, 256) = (b, m)
        out[:, IN_FEATURES + c * B_LOCAL:IN_FEATURES + (c + 1) * B_LOCAL] = blk.T
    return out


def _in_maps(x, T):
    bf16 = ml_dtypes.bfloat16
    xt = np.ascontiguousarray(x.T).astype(bf16)          # (1024, 256)
    selr, i4h, i4v, st4, lmov, wf, blk4, blk4t = _stationaries()
    selr = np.ascontiguousarray(selr.reshape(128, 512)).astype(bf16)
    blk4b = np.ascontiguousarray(blk4.reshape(B_LOCAL, 512)).astype(bf16)
    blk4t = np.ascontiguousarray(blk4t.reshape(128, 128))
    blk4tb = blk4t.astype(bf16)
    i128m = np.eye(128, dtype=np.float32).astype(bf16)
    i4h = i4h.astype(bf16)
    i4v = i4v.astype(bf16)
    st4 = st4.astype(bf16)
    lmov = lmov.astype(bf16)
    in_maps = []
    for c in range(N_CORES):
        # k-major: column = k*32 + b
        tsh = np.ascontiguousarray(
            T[:, c * B_LOCAL:(c + 1) * B_LOCAL, :].transpose(0, 2, 1)
            .reshape(IN_FEATURES, BK)).astype(bf16)
        in_maps.append({"xT": xt, "Tsh": tsh, "SELR": selr,
                        "I4H": i4h, "I4V": i4v, "ST4": st4, "LMOV": lmov,
                        "WF": wf, "BLK4B": blk4b, "BLK4T": blk4t,
                        "BLK4TB": blk4tb, "I128": i128m})
    return in_maps
